# revision 53
# baseline (speedup 1.0000x reference)
"""Trainium2 Bass kernel for nn_BestRqLossNetwork (best-RQ masked-prediction loss).

Math (per the reference):
    logits  = context @ W_enc + b_enc                      # (N,T,K)
    targets = argmin_k ||normalize(feats @ proj) - cb_k||  # == argmax_k (feats@proj)·cb_k
                                                           #    (cb rows unit-norm, row norm > 0)
    loss    = mean over valid (t < lens[n]) of CE(logits, targets)

Distribution: data-parallel over the 8192 (n, t) positions — 1024 consecutive
tokens per core (each core's slab lies inside one sequence since T = 2*1024).
Weights (W_enc, codebook, proj) are replicated. Each core returns per-token
(sum_exp, target_logit, valid) packed as a (128, 24) matrix; the host computes
ln(sum_exp) and the global masked mean (the ln/table-switch and final
reductions are cheaper on host than as a serial device epilogue).

Per-core pipeline, phase p = tile p (128 tokens on partitions), groups g = 0..7
(1024 classes each):
  PE   : scores(p,g) = fT.T @ cbT (contract 16, 2x 512-wide) and
         logits(p,g) = ctxT.T @ W (fp8 DoubleRow, 4x 512-wide) interleaved.
  ACT  : exp with row-sum accumulation (|logits| small; exp cannot overflow).
  DVE  : fused PSUM->SBUF copy + per-1024-chunk max (tensor_scalar accum max);
         two-level argmax (MAX_INDEX over chunk maxes -> indirect-DMA gather
         of the winning 1024-chunk from a DRAM staging buffer -> MAX_INDEX
         within it); target logit = dot(context_row, gathered W_enc.T row).
  POOL : issue gathers, small index arithmetic / reductions (keeps DVE lean).

The argmax chain for tile p runs during phase p+1 (chainA at g1, chainB at g4,
dot at g7), giving every staging/gather DMA ~a-group of latency to land.

The whole schedule is built to keep the Tensor engine's HAM clock at 2.4 GHz:
the PE ramps to full speed after ~3us of continuous work, and any idle gap
drops it to 1.2 GHz for a long hysteresis window (this was measured to cost
the baseline ~2x on most of the kernel). Hence: warm-up matmuls start at ~1us
(memset on the early-starting Pool engine), input DMAs are consolidated so
bulk data streams while the PE warms, and standalone LDWEIGHTS fill the one
unavoidable wait (fT PSUM->SBUF copy) between warm-up and the first phase.
"""

import numpy as np
import ml_dtypes

N, T, F, V, K = 4, 2048, 512, 16, 8192
NCORES = 8
TOK = (N * T) // NCORES   # tokens per core
P = 128                   # partitions / tokens per tile
NTILES = TOK // P         # 8
CC = F // P               # 4 contraction chunks of 128
MC = K // 1024            # 8 mega-chunks of 1024 classes

# res_sb column layout: [0:64) per-(tile,group) exp sums (host reduces the 8
# groups per tile), [64:72) target logit, [72:80) valid
S0 = 0
L0 = NTILES * MC
C0 = L0 + NTILES
RESW = C0 + NTILES

_BF16 = ml_dtypes.bfloat16
_FP8 = ml_dtypes.float8_e4m3
_cache: dict = {}


def build_program(has_bias: bool):
    """Build + compile the single-core Bass program (run SPMD on 8 cores)."""
    from concourse import bacc
    import concourse.bass as bass
    import concourse.tile as tile
    import concourse.mybir as mybir

    dt = mybir.dt
    alu = mybir.AluOpType
    act = mybir.ActivationFunctionType

    nc = bacc.Bacc(
        "TRN2", target_bir_lowering=False, debug=False, num_devices=NCORES
    )

    ctxT = nc.dram_tensor("ctxT", [F, TOK], dt.float8e4, kind="ExternalInput").ap()
    ctx = nc.dram_tensor("ctx", [TOK, F], dt.bfloat16, kind="ExternalInput").ap()
    fTd = nc.dram_tensor("fT", [V, TOK], dt.bfloat16, kind="ExternalInput").ap()
    w = nc.dram_tensor("w", [F, K], dt.float8e4, kind="ExternalInput").ap()
    wt = nc.dram_tensor("wt", [K, F], dt.bfloat16, kind="ExternalInput").ap()
    cbt = nc.dram_tensor("cbt", [V, K], dt.bfloat16, kind="ExternalInput").ap()
    adjlen = nc.dram_tensor("adjlen", [P, 1], dt.float32, kind="ExternalInput").ap()
    tidx = nc.dram_tensor("tidx", [P, 1], dt.float32, kind="ExternalInput").ap()
    tidx_i = nc.dram_tensor("tidx_i", [P, 1], dt.int32, kind="ExternalInput").ap()
    if has_bias:
        brow = nc.dram_tensor("brow", [1, K], dt.bfloat16, kind="ExternalInput").ap()
        bcol = nc.dram_tensor("bcol", [K, 1], dt.float32, kind="ExternalInput").ap()
    res = nc.dram_tensor("res", [P, RESW], dt.float32, kind="ExternalOutput").ap()
    # DRAM staging for the two-level argmax: row (tok*MC + mc) holds that
    # token's mc-th 1024-wide score chunk (bf16).
    stage = nc.dram_tensor("scstage", [TOK * MC, 1024], dt.bfloat16).ap()
    stage_v = stage.rearrange("(t m) k -> t m k", m=MC)

    # 3D views of the DRAM inputs so each bulk load is ONE dma_start.
    w_v = w.rearrange("(c p) k -> p c k", p=P)
    ctxT_v = ctxT.rearrange("(c p) t -> p c t", p=P)
    ctx_v = ctx.rearrange("(j p) f -> p j f", p=P)

    with tile.TileContext(nc) as tc:
        with (
            tc.tile_pool(name="singles", bufs=1) as singles,
            tc.tile_pool(name="work", bufs=3) as work,
            tc.tile_pool(name="stg", bufs=6) as stg,
            tc.tile_pool(name="sc_ps", bufs=2, space="PSUM") as sc_ps_pool,
            tc.tile_pool(name="lg_ps", bufs=2, space="PSUM") as lg_ps_pool,
        ):
            # ---- resident SBUF tensors ----
            w_sb = singles.tile([P, CC, K], dt.float8e4)
            ctxT_sb = singles.tile([P, CC, TOK], dt.float8e4)
            ctx_sb = singles.tile([P, NTILES, F], dt.bfloat16)
            cbt_sb = singles.tile([V, K], dt.bfloat16)
            fT_sb = singles.tile([V, TOK], dt.bfloat16)
            adjlen_sb = singles.tile([P, 1], dt.float32)
            tidx_sb = singles.tile([P, 1], dt.float32)
            tidxi_sb = singles.tile([P, 1], dt.int32)
            warm_sb = singles.tile([P, 512], dt.bfloat16)
            exp_scr = singles.tile([P, 1024], dt.bfloat16)
            res_sb = singles.tile([P, RESW], dt.float32)

            # The Pool engine's queue comes up first (~1us vs ~7us for
            # Vector), so the warm-up matmuls can start almost immediately.
            nc.gpsimd.memset(warm_sb[:, :], 0.0)

            def emit_warm_mm(n=1):
                for _ in range(n):
                    wz = sc_ps_pool.tile([P, 512], dt.float32, tag="sp", name="wz")
                    nc.tensor.matmul(
                        out=wz[:, :], lhsT=warm_sb[:, 0:P], rhs=warm_sb[:, :],
                        start=True, stop=True,
                    )

            def emit_pe_fill(n=1):
                # Dependency-free PE work that consumes no PSUM: keeps the
                # HAM activity monitor from seeing an idle Tensor engine.
                for _ in range(n):
                    nc.tensor.ldweights(warm_sb[:, 0:P])

            def emit_warm_lg(n=1):
                # Warm matmuls into the (phase-0-idle) logits PSUM pool —
                # the scores pool's 2-buffer rotation would stall on the
                # pending DVE copy.
                for _ in range(n):
                    wz = lg_ps_pool.tile([P, 512], dt.float32, tag="lp", name="wzl")
                    nc.tensor.matmul(
                        out=wz[:, :], lhsT=warm_sb[:, 0:P], rhs=warm_sb[:, :],
                        start=True, stop=True,
                    )

            # Warm until the scores-side inputs (fT, cbt q0 — tiny) have
            # landed (~12us). The HAM re-ramp after a down-clock is
            # stochastic (sometimes never), so the warm stream must hand
            # over to real work with NO gap >~0.7us, ever. Phase 0 then runs
            # scores-only (fully fed, warm-fill balanced against the DVE
            # pace) while the 4 MB of w streams in, and its logits run as a
            # back-to-back block afterwards, when w has landed (~25-27us —
            # an empirical constant across runs set by aggregate DMA BW).
            emit_warm_mm(12)

            # Startup loads. One dma_start lands on one DMA engine
            # (~13-35 GB/s depending on contention), so the critical tensors
            # are split into chunks and issued in need-order; fT (32 KB,
            # host-computed feats@proj) makes the scores side ready almost
            # immediately. ctx (needed last) issues mid-phase-0 from sync.
            # Pool queue: fT + first cbt quarter (phase-0 g0), ctxT, misc,
            # rest of cbt, w groups 6-7.
            nc.gpsimd.dma_start(out=fT_sb[:, :], in_=fTd[:, :])
            nc.gpsimd.dma_start(out=cbt_sb[:, 0:2048], in_=cbt[:, 0:2048])
            for cc in range(CC):
                nc.gpsimd.dma_start(out=ctxT_sb[:, cc, :], in_=ctxT_v[:, cc, :])
            for g in (6, 7):
                for hh in range(2):
                    qsl = slice(g * 1024 + hh * 512, g * 1024 + (hh + 1) * 512)
                    nc.gpsimd.dma_start(out=w_sb[:, :, qsl], in_=w_v[:, :, qsl])
            for q in range(1, 4):
                qsl = slice(q * 2048, (q + 1) * 2048)
                nc.gpsimd.dma_start(out=cbt_sb[:, qsl], in_=cbt[:, qsl])
            nc.gpsimd.dma_start(out=adjlen_sb[:, :], in_=adjlen[:, :])
            nc.gpsimd.dma_start(out=tidx_sb[:, :], in_=tidx[:, :])
            nc.gpsimd.dma_start(out=tidxi_sb[:, :], in_=tidx_i[:, :])
            # sync queue: w group 0 in quarters, group 1 halved (staging DMAs
            # start behind these, ~13us).
            for q in range(4):
                qsl = slice(q * 256, (q + 1) * 256)
                nc.sync.dma_start(out=w_sb[:, :, qsl], in_=w_v[:, :, qsl])
            for hh in range(2):
                qsl = slice(1024 + hh * 512, 1024 + (hh + 1) * 512)
                nc.sync.dma_start(out=w_sb[:, :, qsl], in_=w_v[:, :, qsl])
            # ACT queue: w groups 2-5 (exp work starts only ~15us in).
            for g in (2, 3, 4, 5):
                for hh in range(2):
                    qsl = slice(g * 1024 + hh * 512, g * 1024 + (hh + 1) * 512)
                    nc.scalar.dma_start(out=w_sb[:, :, qsl], in_=w_v[:, :, qsl])

            if has_bias:
                onesrow_sb = singles.tile([1, P], dt.bfloat16)
                brow_sb = singles.tile([1, K], dt.bfloat16)
                nc.gpsimd.memset(onesrow_sb[:, :], 1.0)
                nc.scalar.dma_start(out=brow_sb[:, :], in_=brow[:, :])

            # valid mask per tile: (tidx - adjlen) < -128*j  <=>  j*128 + t < len
            for j in range(NTILES):
                nc.gpsimd.tensor_scalar(
                    out=res_sb[:, C0 + j:C0 + j + 1],
                    in0=tidx_sb[:, :],
                    scalar1=adjlen_sb[:, 0:1],
                    scalar2=float(-(j * P)),
                    op0=alu.subtract,
                    op1=alu.is_lt,
                )

            # PE fill inventory bridging the load window into phase 0: the
            # PE's 64-deep reorder window pulls these dependency-free
            # LDWEIGHTS ahead of any data-stalled matmul, so they cover
            # stalls anywhere within ~64 instructions of emission.
            emit_pe_fill(24)

            # ---- per-phase emitters ----
            st = {}  # per-tile live tiles: cmA/m1/mc8/scrow/widx/wrow

            def emit_scores_mega(j, mc):
                """One 1024-wide scores mega-chunk: two matmuls into one PSUM
                tile, a single fused copy+max, then DMA the chunk to DRAM."""
                tsl = slice(j * P, (j + 1) * P)
                s = st.setdefault(j, {})
                if mc == 0:
                    s["cmA"] = work.tile([P, MC], dt.bfloat16, tag="cma",
                                         name=f"cma{j}", bufs=4)
                sstg = stg.tile([P, 1024], dt.bfloat16, tag="sstg",
                                name=f"sstg{j}_{mc}")
                sp = sc_ps_pool.tile([P, 1024], dt.float32, tag="sp")
                for h in range(2):
                    nc.tensor.matmul(
                        out=sp[:, h * 512:(h + 1) * 512],
                        lhsT=fT_sb[:, tsl],
                        rhs=cbt_sb[:, mc * 1024 + h * 512:mc * 1024 + (h + 1) * 512],
                        start=True,
                        stop=True,
                    )
                nc.vector.tensor_scalar(
                    out=sstg[:, :],
                    in0=sp[:, :],
                    scalar1=0.0,
                    scalar2=None,
                    op0=alu.add,
                    op1=alu.max,
                    accum_out=s["cmA"][:, mc:mc + 1],
                )
                nc.sync.dma_start(out=stage_v[tsl, mc, :], in_=sstg[:, :])

            def emit_chainA(j):
                """Level-1 argmax over chunk maxes + issue the score-chunk
                gather (staging DMAs got ~a group's latency to land)."""
                s = st[j]
                cm = s["cmA"]
                m1 = work.tile([P, 1], dt.bfloat16, tag="m1", name=f"m1_{j}")
                nc.vector.tensor_reduce(
                    out=m1[:, :], in_=cm[:, :], axis=mybir.AxisListType.X, op=alu.max
                )
                mc8 = work.tile([P, 8], dt.uint32, tag="mc8", name=f"mc8_{j}")
                nc.vector.max_index(mc8[:, :], m1[:, 0:1].to_broadcast([P, 8]), cm[:, :])
                rowid = work.tile([P, 1], dt.int32, tag="rowid", name=f"rid{j}")
                nc.gpsimd.tensor_scalar(
                    out=rowid[:, :], in0=tidxi_sb[:, :],
                    scalar1=float(MC), scalar2=float(j * P * MC),
                    op0=alu.mult, op1=alu.add,
                )
                nc.gpsimd.tensor_tensor(
                    out=rowid[:, :], in0=rowid[:, :],
                    in1=mc8[:, 0:1].bitcast(dt.int32), op=alu.add,
                )
                scrow = work.tile([P, 1024], dt.bfloat16, tag="scrow",
                                  name=f"scrow{j}")
                nc.gpsimd.indirect_dma_start(
                    out=scrow[:, :],
                    out_offset=None,
                    in_=stage[:, :],
                    in_offset=bass.IndirectOffsetOnAxis(ap=rowid[:, 0:1], axis=0),
                )
                s["m1"], s["mc8"], s["scrow"] = m1, mc8, scrow

            def emit_chainB(j):
                """Level-2 argmax within the gathered chunk + issue the
                W_enc.T row gather."""
                s = st[j]
                l2i = work.tile([P, 8], dt.uint32, tag="l2i", name=f"l2i{j}")
                nc.vector.max_index(
                    l2i[:, :], s["m1"][:, 0:1].to_broadcast([P, 8]), s["scrow"][:, :]
                )
                widx = work.tile([P, 1], dt.int32, tag="widx", name=f"widx{j}")
                nc.gpsimd.tensor_scalar(
                    out=widx[:, :], in0=s["mc8"][:, 0:1].bitcast(dt.int32),
                    scalar1=1024.0, scalar2=None, op0=alu.mult,
                )
                nc.gpsimd.tensor_tensor(
                    out=widx[:, :], in0=widx[:, :],
                    in1=l2i[:, 0:1].bitcast(dt.int32), op=alu.add,
                )
                wrow = work.tile([P, F], dt.bfloat16, tag="wrow", name=f"wrow{j}")
                nc.gpsimd.indirect_dma_start(
                    out=wrow[:, :],
                    out_offset=None,
                    in_=wt[:, :],
                    in_offset=bass.IndirectOffsetOnAxis(ap=widx[:, 0:1], axis=0),
                )
                s["wrow"] = wrow
                if has_bias:
                    bg = work.tile([P, 1], dt.float32, tag="bg", name=f"bg{j}")
                    nc.gpsimd.indirect_dma_start(
                        out=bg[:, :],
                        out_offset=None,
                        in_=bcol[:, :],
                        in_offset=bass.IndirectOffsetOnAxis(ap=widx[:, 0:1], axis=0),
                    )
                    s["bg"] = bg

            def emit_dot_mul(j):
                """Elementwise ctx_row * W_row on the Pool engine (both
                operands are SBUF) to keep the DVE free for copies."""
                s = st[j]
                prod = work.tile([P, F], dt.bfloat16, tag="prod", name=f"prod{j}")
                nc.gpsimd.tensor_tensor(
                    out=prod[:, :], in0=ctx_sb[:, j, :], in1=s["wrow"][:, :],
                    op=alu.mult,
                )
                s["prod"] = prod

            def emit_dot_red(j):
                """Target logit: free-dim sum of the products (DVE)."""
                s = st[j]
                nc.vector.tensor_reduce(
                    out=res_sb[:, L0 + j:L0 + j + 1], in_=s["prod"][:, :],
                    axis=mybir.AxisListType.X, op=alu.add,
                )
                if has_bias:
                    nc.gpsimd.tensor_tensor(
                        out=res_sb[:, L0 + j:L0 + j + 1],
                        in0=res_sb[:, L0 + j:L0 + j + 1],
                        in1=s["bg"][:, :], op=alu.add,
                    )
                del st[j]

            def emit_logits_group(j, g, sums):
                tsl = slice(j * P, (j + 1) * P)
                lp = lg_ps_pool.tile([P, 1024], dt.float32, tag="lp")
                for h in range(2):
                    hsl = slice(h * 512, (h + 1) * 512)
                    for cc2 in range(0, CC, 2):
                        nc.tensor.matmul(
                            out=lp[:, hsl],
                            lhsT=ctxT_sb[:, cc2:cc2 + 2, tsl],
                            rhs=w_sb[:, cc2:cc2 + 2, g * 1024 + h * 512:g * 1024 + (h + 1) * 512],
                            start=(cc2 == 0),
                            stop=(cc2 == CC - 2 and not has_bias),
                            perf_mode=mybir.MatmulPerfMode.DoubleRow,
                        )
                    if has_bias:
                        nc.tensor.matmul(
                            out=lp[:, hsl],
                            lhsT=onesrow_sb[:, :],
                            rhs=brow_sb[:, g * 1024 + h * 512:g * 1024 + (h + 1) * 512],
                            start=False,
                            stop=True,
                        )
                nc.scalar.activation(
                    out=exp_scr[:, :],
                    in_=lp[:, :],
                    func=act.Exp,
                    scale=1.0 / 64.0,
                    accum_out=sums[:, g:g + 1],
                )

            # ---- uniform software-pipelined phases ----
            # Per phase p: stream scores(p) + logits(p). The argmax chain runs
            # DEPTH-2 (chainA for tile p-1 @g1; chainB/dot for tile p-2 @g2+)
            # so every gather has phase-scale latency margin — an in-order DVE
            # queue means any instruction waiting on a late gather blocks all
            # later copies and stalls the PE (observed: scrow gathers jitter
            # by several us under staging-DMA contention).
            # The LAST tile's scores are front-loaded (2 per group, g0-g3) so
            # its chain overlaps the remaining logits instead of running as a
            # serial post-loop tail.
            last = NTILES - 1
            # The pipeline is SKEWED one block: block b streams scores(b)
            # together with logits(b-1). Block 0 (scores only — w hasn't
            # landed) is DVE-copy-paced with warm-matmul top-up; blocks 1-7
            # are dense on PE/DVE/ACT alike; block 8 (logits(7) only) is the
            # single exp-paced low-utilization stretch, parked at the END
            # where a power down-clock can no longer hurt (everything after
            # it is DVE/Pool/DMA tail work). The skew also gives w a whole
            # extra block to land before the first logits group needs it.
            for b in range(NTILES + 1):
                if b == 1:
                    # Jitter shield: on straggler cores w group 0 can land
                    # ~4-6us after block 0's scores finish; idle there costs
                    # a (possibly unrecoverable) down-clock.
                    emit_warm_lg(10)
                for g in range(MC):
                    if b < NTILES:
                        emit_scores_mega(b, g)
                        if b == 0:
                            emit_warm_lg(2)
                            if g == 3:
                                # ctx is needed from block 2 g5 on; issue it
                                # from the (otherwise quiet) sync queue.
                                for jj in range(0, NTILES, 2):
                                    nc.sync.dma_start(
                                        out=ctx_sb[:, jj:jj + 2, :],
                                        in_=ctx_v[:, jj:jj + 2, :],
                                    )
                    if b >= 1:
                        sums = res_sb[:, S0 + (b - 1) * MC:S0 + b * MC]
                        emit_logits_group(b - 1, g, sums)
                        if g == 1:
                            emit_chainA(b - 1)
                    if b >= 2:
                        if g == 2:
                            emit_chainB(b - 2)
                        elif g == 5:
                            emit_dot_mul(b - 2)
                        elif g == 7:
                            emit_dot_red(b - 2)

            # ---- tail: tile 7's chain (gathers already in flight) ----
            emit_chainB(last)
            emit_dot_mul(last)
            emit_dot_red(last)
            nc.sync.dma_start(out=res[:, :], in_=res_sb[:, :])

    nc.compile()
    return nc


def _get_program(has_bias: bool):
    if has_bias not in _cache:
        _cache[has_bias] = build_program(has_bias)
    return _cache[has_bias]


def make_in_maps(feats, context, lens, proj_matrix, codebook, W_enc, b_enc,
                 has_bias):
    """Shard + lay out the full inputs into per-core input maps."""
    feats_f = np.ascontiguousarray(feats).reshape(N * T, F)
    ctx_f = np.ascontiguousarray(context).reshape(N * T, F)
    w_f8 = (W_enc * 64.0).astype(_FP8)
    wt_bf = np.ascontiguousarray(W_enc.T).astype(_BF16)
    cbt_bf = np.ascontiguousarray(codebook.T).astype(_BF16)
    # f = feats @ proj, unnormalized: the row norm is positive, so the
    # argmax over codebook entries is unchanged (codebook rows unit-norm).
    f_all = feats_f.astype(np.float32) @ proj_matrix.astype(np.float32)
    tidx_a = np.arange(P, dtype=np.float32).reshape(P, 1)
    tidx_ia = np.arange(P, dtype=np.int32).reshape(P, 1)

    in_maps = []
    for c in range(NCORES):
        sl = slice(c * TOK, (c + 1) * TOK)
        ctxs = ctx_f[sl]
        n_idx = (c * TOK) // T
        t_off = (c * TOK) % T
        adj = np.full((P, 1), float(int(lens[n_idx]) - t_off), dtype=np.float32)
        m = {
            "ctxT": np.ascontiguousarray(ctxs.T).astype(_FP8),
            "ctx": ctxs.astype(_BF16),
            "fT": np.ascontiguousarray(f_all[sl].T).astype(_BF16),
            "w": w_f8,
            "wt": wt_bf,
            "cbt": cbt_bf,
            "adjlen": adj,
            "tidx": tidx_a,
            "tidx_i": tidx_ia,
        }
        if has_bias:
            m["brow"] = np.ascontiguousarray(b_enc * 64.0).reshape(1, K).astype(_BF16)
            m["bcol"] = np.ascontiguousarray(b_enc).reshape(K, 1).astype(np.float32)
        in_maps.append(m)
    return in_maps


def kernel(feats, context, lens, proj_matrix, codebook, W_enc, b_enc,
           _want_results=False, _trace=False):
    from concourse.bass_utils import run_bass_kernel_spmd

    has_bias = bool(np.any(np.asarray(b_enc) != 0))
    nc = _get_program(has_bias)
    in_maps = make_in_maps(feats, context, lens, proj_matrix, codebook, W_enc,
                           b_enc, has_bias)
    res = run_bass_kernel_spmd(
        nc, in_maps, list(range(NCORES)), trace=_trace,
        trace_cores=list(range(NCORES)) if _trace else None,
    )
    num = 0.0
    den = 0.0
    for r in res.results:
        m = np.asarray(r["res"], dtype=np.float64)  # (128, 80)
        s = m[:, S0:L0].reshape(P, NTILES, MC).sum(axis=2)
        lt, cnt = m[:, L0:C0], m[:, C0:]
        num += float((cnt * (np.log(s) - lt)).sum())
        den += float(cnt.sum())
    loss = np.array(np.float32(num / max(den, 1.0)))
    if _want_results:
        return loss, res
    return loss


# revision 54
# speedup vs baseline: 1.2088x; 1.2088x over previous
"""Trainium2 Bass kernel for nn_BestRqLossNetwork (best-RQ masked-prediction loss).

Math (per the reference):
    logits  = context @ W_enc + b_enc                      # (N,T,K)
    targets = argmin_k ||normalize(feats @ proj) - cb_k||  # == argmax_k (feats@proj)·cb_k
                                                           #    (cb rows unit-norm, row norm > 0)
    loss    = mean over valid (t < lens[n]) of CE(logits, targets)

Distribution: data-parallel over the 8192 (n, t) positions — 1024 consecutive
tokens per core (each core's slab lies inside one sequence since T = 2*1024).
Weights (W_enc, codebook, proj) are replicated. Each core returns per-token
(sum_exp, target_logit, valid) packed as a (128, 24) matrix; the host computes
ln(sum_exp) and the global masked mean (the ln/table-switch and final
reductions are cheaper on host than as a serial device epilogue).

Per-core pipeline, phase p = tile p (128 tokens on partitions), groups g = 0..7
(1024 classes each):
  PE   : scores(p,g) = fT.T @ cbT (contract 16, 2x 512-wide) and
         logits(p,g) = ctxT.T @ W (fp8 DoubleRow, 4x 512-wide) interleaved.
  ACT  : exp with row-sum accumulation (|logits| small; exp cannot overflow).
  DVE  : fused PSUM->SBUF copy + per-1024-chunk max (tensor_scalar accum max);
         two-level argmax (MAX_INDEX over chunk maxes -> indirect-DMA gather
         of the winning 1024-chunk from a DRAM staging buffer -> MAX_INDEX
         within it); target logit = dot(context_row, gathered W_enc.T row).
  POOL : issue gathers, small index arithmetic / reductions (keeps DVE lean).

The argmax chain for tile p runs during phase p+1 (chainA at g1, chainB at g4,
dot at g7), giving every staging/gather DMA ~a-group of latency to land.

The whole schedule is built to keep the Tensor engine's HAM clock at 2.4 GHz:
the PE ramps to full speed after ~3us of continuous work, and any idle gap
drops it to 1.2 GHz for a long hysteresis window (this was measured to cost
the baseline ~2x on most of the kernel). Hence: warm-up matmuls start at ~1us
(memset on the early-starting Pool engine), input DMAs are consolidated so
bulk data streams while the PE warms, and standalone LDWEIGHTS fill the one
unavoidable wait (fT PSUM->SBUF copy) between warm-up and the first phase.
"""

import numpy as np
import ml_dtypes

N, T, F, V, K = 4, 2048, 512, 16, 8192
NCORES = 8
TOK = (N * T) // NCORES   # tokens per core
P = 128                   # partitions / tokens per tile
NTILES = TOK // P         # 8
CC = F // P               # 4 contraction chunks of 128
MC = K // 1024            # 8 mega-chunks of 1024 classes

# res_sb column layout: [0:64) per-(tile,group) exp sums (host reduces the 8
# groups per tile), [64:72) target logit, [72:80) valid
S0 = 0
L0 = NTILES * MC
C0 = L0 + NTILES
RESW = C0 + NTILES

_BF16 = ml_dtypes.bfloat16
_FP8 = ml_dtypes.float8_e4m3
_cache: dict = {}


def build_program(has_bias: bool):
    """Build + compile the single-core Bass program (run SPMD on 8 cores)."""
    from concourse import bacc
    import concourse.bass as bass
    import concourse.tile as tile
    import concourse.mybir as mybir

    dt = mybir.dt
    alu = mybir.AluOpType
    act = mybir.ActivationFunctionType

    nc = bacc.Bacc(
        "TRN2", target_bir_lowering=False, debug=False, num_devices=NCORES
    )

    ctxT = nc.dram_tensor("ctxT", [F, TOK], dt.float8e4, kind="ExternalInput").ap()
    ctx = nc.dram_tensor("ctx", [TOK, F], dt.bfloat16, kind="ExternalInput").ap()
    fTd = nc.dram_tensor("fT", [V, TOK], dt.bfloat16, kind="ExternalInput").ap()
    w = nc.dram_tensor("w", [F, K], dt.float8e4, kind="ExternalInput").ap()
    wt = nc.dram_tensor("wt", [K, F], dt.bfloat16, kind="ExternalInput").ap()
    cbt = nc.dram_tensor("cbt", [V, K], dt.bfloat16, kind="ExternalInput").ap()
    adjlen = nc.dram_tensor("adjlen", [P, 1], dt.float32, kind="ExternalInput").ap()
    tidx = nc.dram_tensor("tidx", [P, 1], dt.float32, kind="ExternalInput").ap()
    tidx_i = nc.dram_tensor("tidx_i", [P, 1], dt.int32, kind="ExternalInput").ap()
    if has_bias:
        brow = nc.dram_tensor("brow", [1, K], dt.bfloat16, kind="ExternalInput").ap()
        bcol = nc.dram_tensor("bcol", [K, 1], dt.float32, kind="ExternalInput").ap()
    res = nc.dram_tensor("res", [P, RESW], dt.float32, kind="ExternalOutput").ap()
    # DRAM staging for the two-level argmax: row (tok*MC + mc) holds that
    # token's mc-th 1024-wide score chunk (bf16).
    stage = nc.dram_tensor("scstage", [TOK * MC, 1024], dt.bfloat16).ap()
    stage_v = stage.rearrange("(t m) k -> t m k", m=MC)

    # 3D views of the DRAM inputs so each bulk load is ONE dma_start.
    w_v = w.rearrange("(c p) k -> p c k", p=P)
    ctxT_v = ctxT.rearrange("(c p) t -> p c t", p=P)
    ctx_v = ctx.rearrange("(j p) f -> p j f", p=P)

    with tile.TileContext(nc) as tc:
        with (
            tc.tile_pool(name="singles", bufs=1) as singles,
            tc.tile_pool(name="work", bufs=3) as work,
            tc.tile_pool(name="stg", bufs=6) as stg,
            tc.tile_pool(name="sc_ps", bufs=2, space="PSUM") as sc_ps_pool,
            tc.tile_pool(name="lg_ps", bufs=2, space="PSUM") as lg_ps_pool,
        ):
            # ---- resident SBUF tensors ----
            w_sb = singles.tile([P, CC, K], dt.float8e4)
            ctxT_sb = singles.tile([P, CC, TOK], dt.float8e4)
            ctx_sb = singles.tile([P, NTILES, F], dt.bfloat16)
            cbt_sb = singles.tile([V, K], dt.bfloat16)
            fT_sb = singles.tile([V, TOK], dt.bfloat16)
            adjlen_sb = singles.tile([P, 1], dt.float32)
            tidx_sb = singles.tile([P, 1], dt.float32)
            tidxi_sb = singles.tile([P, 1], dt.int32)
            warm_sb = singles.tile([P, 512], dt.bfloat16)
            exp_scr = singles.tile([P, 1024], dt.bfloat16)
            res_sb = singles.tile([P, RESW], dt.float32)

            # The Pool engine's queue comes up first (~1us vs ~7us for
            # Vector), so the warm-up matmuls can start almost immediately.
            nc.gpsimd.memset(warm_sb[:, :], 0.0)

            def emit_warm_mm(n=1):
                for _ in range(n):
                    wz = sc_ps_pool.tile([P, 512], dt.float32, tag="sp", name="wz")
                    nc.tensor.matmul(
                        out=wz[:, :], lhsT=warm_sb[:, 0:P], rhs=warm_sb[:, :],
                        start=True, stop=True,
                    )

            def emit_pe_fill(n=1):
                # Dependency-free PE work that consumes no PSUM: keeps the
                # HAM activity monitor from seeing an idle Tensor engine.
                for _ in range(n):
                    nc.tensor.ldweights(warm_sb[:, 0:P])

            def emit_warm_lg(n=1):
                # Warm matmuls into the (phase-0-idle) logits PSUM pool —
                # the scores pool's 2-buffer rotation would stall on the
                # pending DVE copy.
                for _ in range(n):
                    wz = lg_ps_pool.tile([P, 512], dt.float32, tag="lp", name="wzl")
                    nc.tensor.matmul(
                        out=wz[:, :], lhsT=warm_sb[:, 0:P], rhs=warm_sb[:, :],
                        start=True, stop=True,
                    )

            # Warm until the scores-side inputs (fT, cbt q0 — tiny) have
            # landed (~12us). The HAM re-ramp after a down-clock is
            # stochastic (sometimes never), so the warm stream must hand
            # over to real work with NO gap >~0.7us, ever. Phase 0 then runs
            # scores-only (fully fed, warm-fill balanced against the DVE
            # pace) while the 4 MB of w streams in, and its logits run as a
            # back-to-back block afterwards, when w has landed (~25-27us —
            # an empirical constant across runs set by aggregate DMA BW).
            emit_warm_mm(12)

            # Startup loads. One dma_start lands on one DMA engine
            # (~13-35 GB/s depending on contention), so the critical tensors
            # are split into chunks and issued in need-order; fT (32 KB,
            # host-computed feats@proj) makes the scores side ready almost
            # immediately. ctx (needed last) issues mid-phase-0 from sync.
            # Pool queue: fT + first cbt quarter (phase-0 g0), ctxT, misc,
            # rest of cbt, w groups 6-7.
            nc.gpsimd.dma_start(out=fT_sb[:, :], in_=fTd[:, :])
            nc.gpsimd.dma_start(out=cbt_sb[:, 0:2048], in_=cbt[:, 0:2048])
            for cc in range(CC):
                nc.gpsimd.dma_start(out=ctxT_sb[:, cc, :], in_=ctxT_v[:, cc, :])
            for g in (6, 7):
                for hh in range(2):
                    qsl = slice(g * 1024 + hh * 512, g * 1024 + (hh + 1) * 512)
                    nc.gpsimd.dma_start(out=w_sb[:, :, qsl], in_=w_v[:, :, qsl])
            for q in range(1, 4):
                qsl = slice(q * 2048, (q + 1) * 2048)
                nc.gpsimd.dma_start(out=cbt_sb[:, qsl], in_=cbt[:, qsl])
            nc.gpsimd.dma_start(out=adjlen_sb[:, :], in_=adjlen[:, :])
            nc.gpsimd.dma_start(out=tidx_sb[:, :], in_=tidx[:, :])
            nc.gpsimd.dma_start(out=tidxi_sb[:, :], in_=tidx_i[:, :])
            # sync queue: w group 0 in quarters, group 1 halved (staging DMAs
            # start behind these, ~13us).
            for q in range(4):
                qsl = slice(q * 256, (q + 1) * 256)
                nc.sync.dma_start(out=w_sb[:, :, qsl], in_=w_v[:, :, qsl])
            for hh in range(2):
                qsl = slice(1024 + hh * 512, 1024 + (hh + 1) * 512)
                nc.sync.dma_start(out=w_sb[:, :, qsl], in_=w_v[:, :, qsl])
            # ACT queue: w groups 2-5 (exp work starts only ~15us in).
            for g in (2, 3, 4, 5):
                for hh in range(2):
                    qsl = slice(g * 1024 + hh * 512, g * 1024 + (hh + 1) * 512)
                    nc.scalar.dma_start(out=w_sb[:, :, qsl], in_=w_v[:, :, qsl])

            if has_bias:
                onesrow_sb = singles.tile([1, P], dt.bfloat16)
                brow_sb = singles.tile([1, K], dt.bfloat16)
                nc.gpsimd.memset(onesrow_sb[:, :], 1.0)
                nc.scalar.dma_start(out=brow_sb[:, :], in_=brow[:, :])

            # valid mask per tile: (tidx - adjlen) < -128*j  <=>  j*128 + t < len
            for j in range(NTILES):
                nc.gpsimd.tensor_scalar(
                    out=res_sb[:, C0 + j:C0 + j + 1],
                    in0=tidx_sb[:, :],
                    scalar1=adjlen_sb[:, 0:1],
                    scalar2=float(-(j * P)),
                    op0=alu.subtract,
                    op1=alu.is_lt,
                )

            # PE fill inventory bridging the load window into phase 0: the
            # PE's 64-deep reorder window pulls these dependency-free
            # LDWEIGHTS ahead of any data-stalled matmul, so they cover
            # stalls anywhere within ~64 instructions of emission.
            emit_pe_fill(24)

            # ---- per-phase emitters ----
            st = {}  # per-tile live tiles: cmA/m1/mc8/scrow/widx/wrow

            def emit_scores_mega(j, mc):
                """One 1024-wide scores mega-chunk: two matmuls into one PSUM
                tile, a single fused copy+max, then DMA the chunk to DRAM."""
                tsl = slice(j * P, (j + 1) * P)
                s = st.setdefault(j, {})
                if mc == 0:
                    s["cmA"] = work.tile([P, MC], dt.bfloat16, tag="cma",
                                         name=f"cma{j}", bufs=4)
                sstg = stg.tile([P, 1024], dt.bfloat16, tag="sstg",
                                name=f"sstg{j}_{mc}")
                sp = sc_ps_pool.tile([P, 1024], dt.float32, tag="sp")
                for h in range(2):
                    nc.tensor.matmul(
                        out=sp[:, h * 512:(h + 1) * 512],
                        lhsT=fT_sb[:, tsl],
                        rhs=cbt_sb[:, mc * 1024 + h * 512:mc * 1024 + (h + 1) * 512],
                        start=True,
                        stop=True,
                    )
                nc.vector.tensor_scalar(
                    out=sstg[:, :],
                    in0=sp[:, :],
                    scalar1=0.0,
                    scalar2=None,
                    op0=alu.add,
                    op1=alu.max,
                    accum_out=s["cmA"][:, mc:mc + 1],
                )
                nc.sync.dma_start(out=stage_v[tsl, mc, :], in_=sstg[:, :])

            def emit_chainA(j):
                """Level-1 argmax over chunk maxes + issue the score-chunk
                gather (staging DMAs got ~a group's latency to land)."""
                s = st[j]
                cm = s["cmA"]
                m1 = work.tile([P, 1], dt.bfloat16, tag="m1", name=f"m1_{j}")
                nc.vector.tensor_reduce(
                    out=m1[:, :], in_=cm[:, :], axis=mybir.AxisListType.X, op=alu.max
                )
                mc8 = work.tile([P, 8], dt.uint32, tag="mc8", name=f"mc8_{j}")
                nc.vector.max_index(mc8[:, :], m1[:, 0:1].to_broadcast([P, 8]), cm[:, :])
                rowid = work.tile([P, 1], dt.int32, tag="rowid", name=f"rid{j}")
                nc.gpsimd.tensor_scalar(
                    out=rowid[:, :], in0=tidxi_sb[:, :],
                    scalar1=float(MC), scalar2=float(j * P * MC),
                    op0=alu.mult, op1=alu.add,
                )
                nc.gpsimd.tensor_tensor(
                    out=rowid[:, :], in0=rowid[:, :],
                    in1=mc8[:, 0:1].bitcast(dt.int32), op=alu.add,
                )
                scrow = work.tile([P, 1024], dt.bfloat16, tag="scrow",
                                  name=f"scrow{j}")
                nc.gpsimd.indirect_dma_start(
                    out=scrow[:, :],
                    out_offset=None,
                    in_=stage[:, :],
                    in_offset=bass.IndirectOffsetOnAxis(ap=rowid[:, 0:1], axis=0),
                )
                s["m1"], s["mc8"], s["scrow"] = m1, mc8, scrow

            def emit_chainB(j):
                """Level-2 argmax within the gathered chunk + issue the
                W_enc.T row gather."""
                s = st[j]
                l2i = work.tile([P, 8], dt.uint32, tag="l2i", name=f"l2i{j}")
                nc.vector.max_index(
                    l2i[:, :], s["m1"][:, 0:1].to_broadcast([P, 8]), s["scrow"][:, :]
                )
                widx = work.tile([P, 1], dt.int32, tag="widx", name=f"widx{j}")
                nc.gpsimd.tensor_scalar(
                    out=widx[:, :], in0=s["mc8"][:, 0:1].bitcast(dt.int32),
                    scalar1=1024.0, scalar2=None, op0=alu.mult,
                )
                nc.gpsimd.tensor_tensor(
                    out=widx[:, :], in0=widx[:, :],
                    in1=l2i[:, 0:1].bitcast(dt.int32), op=alu.add,
                )
                wrow = work.tile([P, F], dt.bfloat16, tag="wrow", name=f"wrow{j}")
                nc.gpsimd.indirect_dma_start(
                    out=wrow[:, :],
                    out_offset=None,
                    in_=wt[:, :],
                    in_offset=bass.IndirectOffsetOnAxis(ap=widx[:, 0:1], axis=0),
                )
                s["wrow"] = wrow
                if has_bias:
                    bg = work.tile([P, 1], dt.float32, tag="bg", name=f"bg{j}")
                    nc.gpsimd.indirect_dma_start(
                        out=bg[:, :],
                        out_offset=None,
                        in_=bcol[:, :],
                        in_offset=bass.IndirectOffsetOnAxis(ap=widx[:, 0:1], axis=0),
                    )
                    s["bg"] = bg

            def emit_dot_mul(j):
                """Elementwise ctx_row * W_row on the Pool engine (both
                operands are SBUF) to keep the DVE free for copies."""
                s = st[j]
                prod = work.tile([P, F], dt.bfloat16, tag="prod", name=f"prod{j}")
                nc.gpsimd.tensor_tensor(
                    out=prod[:, :], in0=ctx_sb[:, j, :], in1=s["wrow"][:, :],
                    op=alu.mult,
                )
                s["prod"] = prod

            def emit_dot_red(j):
                """Target logit: free-dim sum of the products (DVE)."""
                s = st[j]
                nc.vector.tensor_reduce(
                    out=res_sb[:, L0 + j:L0 + j + 1], in_=s["prod"][:, :],
                    axis=mybir.AxisListType.X, op=alu.add,
                )
                if has_bias:
                    nc.gpsimd.tensor_tensor(
                        out=res_sb[:, L0 + j:L0 + j + 1],
                        in0=res_sb[:, L0 + j:L0 + j + 1],
                        in1=s["bg"][:, :], op=alu.add,
                    )
                del st[j]

            def emit_logits_group(j, g, sums):
                tsl = slice(j * P, (j + 1) * P)
                lp = lg_ps_pool.tile([P, 1024], dt.float32, tag="lp")
                for h in range(2):
                    hsl = slice(h * 512, (h + 1) * 512)
                    for cc2 in range(0, CC, 2):
                        nc.tensor.matmul(
                            out=lp[:, hsl],
                            lhsT=ctxT_sb[:, cc2:cc2 + 2, tsl],
                            rhs=w_sb[:, cc2:cc2 + 2, g * 1024 + h * 512:g * 1024 + (h + 1) * 512],
                            start=(cc2 == 0),
                            stop=(cc2 == CC - 2 and not has_bias),
                            perf_mode=mybir.MatmulPerfMode.DoubleRow,
                        )
                    if has_bias:
                        nc.tensor.matmul(
                            out=lp[:, hsl],
                            lhsT=onesrow_sb[:, :],
                            rhs=brow_sb[:, g * 1024 + h * 512:g * 1024 + (h + 1) * 512],
                            start=False,
                            stop=True,
                        )
                nc.scalar.activation(
                    out=exp_scr[:, :],
                    in_=lp[:, :],
                    func=act.Exp,
                    scale=1.0 / 64.0,
                    accum_out=sums[:, g:g + 1],
                )

            # ---- uniform software-pipelined phases ----
            # Per phase p: stream scores(p) + logits(p). The argmax chain runs
            # DEPTH-2 (chainA for tile p-1 @g1; chainB/dot for tile p-2 @g2+)
            # so every gather has phase-scale latency margin — an in-order DVE
            # queue means any instruction waiting on a late gather blocks all
            # later copies and stalls the PE (observed: scrow gathers jitter
            # by several us under staging-DMA contention).
            # The LAST tile's scores are front-loaded (2 per group, g0-g3) so
            # its chain overlaps the remaining logits instead of running as a
            # serial post-loop tail.
            last = NTILES - 1
            for p in range(NTILES):
                if p >= 1:
                    # Bridge the PE over the phase-boundary DVE lag.
                    emit_pe_fill(6)
                sums = res_sb[:, S0 + p * MC:S0 + (p + 1) * MC]
                if p == 0:
                    # Scores first (their 0.43us/group PE load is topped up
                    # to the 1.22us DVE copy pace with warm matmuls — real
                    # MAC activity for the HAM, unlike bare LDWEIGHTS)...
                    for g in range(MC):
                        emit_scores_mega(p, g)
                        emit_warm_lg(2)
                        if g == 3:
                            # ctx is needed from phase 1 g7 on; issue it
                            # from the (by now otherwise quiet) sync queue.
                            for jj in range(0, NTILES, 2):
                                nc.sync.dma_start(
                                    out=ctx_sb[:, jj:jj + 2, :],
                                    in_=ctx_v[:, jj:jj + 2, :],
                                )
                    # ...then the logits block, consuming w groups in their
                    # DMA arrival order. The warm block in front absorbs
                    # per-core w-arrival jitter (observed up to ~6us on
                    # straggler cores).
                    emit_warm_lg(9)
                    for g in range(MC):
                        emit_logits_group(p, g, sums)
                    continue
                for g in range(MC):
                    emit_scores_mega(p, g)
                    emit_logits_group(p, g, sums)
                    if p >= 1 and g == 1:
                        emit_chainA(p - 1)
                    if p >= 2:
                        if g == 2:
                            emit_chainB(p - 2)
                        elif g == 5:
                            emit_dot_mul(p - 2)
                        elif g == 7:
                            emit_dot_red(p - 2)

            # ---- tail: tiles 6 and 7 chains (gathers already in flight) ----
            emit_chainB(last - 1)
            emit_chainA(last)
            emit_dot_mul(last - 1)
            emit_dot_red(last - 1)
            emit_chainB(last)
            emit_dot_mul(last)
            emit_dot_red(last)
            nc.sync.dma_start(out=res[:, :], in_=res_sb[:, :])

    nc.compile()
    return nc


def _get_program(has_bias: bool):
    if has_bias not in _cache:
        _cache[has_bias] = build_program(has_bias)
    return _cache[has_bias]


def make_in_maps(feats, context, lens, proj_matrix, codebook, W_enc, b_enc,
                 has_bias):
    """Shard + lay out the full inputs into per-core input maps."""
    feats_f = np.ascontiguousarray(feats).reshape(N * T, F)
    ctx_f = np.ascontiguousarray(context).reshape(N * T, F)
    w_f8 = (W_enc * 64.0).astype(_FP8)
    wt_bf = np.ascontiguousarray(W_enc.T).astype(_BF16)
    cbt_bf = np.ascontiguousarray(codebook.T).astype(_BF16)
    # f = feats @ proj, unnormalized: the row norm is positive, so the
    # argmax over codebook entries is unchanged (codebook rows unit-norm).
    f_all = feats_f.astype(np.float32) @ proj_matrix.astype(np.float32)
    tidx_a = np.arange(P, dtype=np.float32).reshape(P, 1)
    tidx_ia = np.arange(P, dtype=np.int32).reshape(P, 1)

    in_maps = []
    for c in range(NCORES):
        sl = slice(c * TOK, (c + 1) * TOK)
        ctxs = ctx_f[sl]
        n_idx = (c * TOK) // T
        t_off = (c * TOK) % T
        adj = np.full((P, 1), float(int(lens[n_idx]) - t_off), dtype=np.float32)
        m = {
            "ctxT": np.ascontiguousarray(ctxs.T).astype(_FP8),
            "ctx": ctxs.astype(_BF16),
            "fT": np.ascontiguousarray(f_all[sl].T).astype(_BF16),
            "w": w_f8,
            "wt": wt_bf,
            "cbt": cbt_bf,
            "adjlen": adj,
            "tidx": tidx_a,
            "tidx_i": tidx_ia,
        }
        if has_bias:
            m["brow"] = np.ascontiguousarray(b_enc * 64.0).reshape(1, K).astype(_BF16)
            m["bcol"] = np.ascontiguousarray(b_enc).reshape(K, 1).astype(np.float32)
        in_maps.append(m)
    return in_maps


def kernel(feats, context, lens, proj_matrix, codebook, W_enc, b_enc,
           _want_results=False, _trace=False):
    from concourse.bass_utils import run_bass_kernel_spmd

    has_bias = bool(np.any(np.asarray(b_enc) != 0))
    nc = _get_program(has_bias)
    in_maps = make_in_maps(feats, context, lens, proj_matrix, codebook, W_enc,
                           b_enc, has_bias)
    res = run_bass_kernel_spmd(
        nc, in_maps, list(range(NCORES)), trace=_trace,
        trace_cores=list(range(NCORES)) if _trace else None,
    )
    num = 0.0
    den = 0.0
    for r in res.results:
        m = np.asarray(r["res"], dtype=np.float64)  # (128, 80)
        s = m[:, S0:L0].reshape(P, NTILES, MC).sum(axis=2)
        lt, cnt = m[:, L0:C0], m[:, C0:]
        num += float((cnt * (np.log(s) - lt)).sum())
        den += float(cnt.sum())
    loss = np.array(np.float32(num / max(den, 1.0)))
    if _want_results:
        return loss, res
    return loss
